# revision 7
# baseline (speedup 1.0000x reference)
"""Adversarial loss kernel for Trainium2 (8 NeuronCores, data-parallel).

For pred [4096, 32000] f32 and target [4096] int:
    out[b] = -(sum_c log(sigmoid(pred[b,c])) - log(sigmoid(pred[b,target[b]]))) / C

Sharding: pure data parallel over the batch dim — 512 rows per core.

Per-core pipeline (memory-bound problem; ~65.5 MB of pred per core):
  1. DMA [128, CT] tiles of pred into SBUF.  The sync HWDGE queue carries
     ONLY these bulk tiles: every [P,1]-shaped transfer (index loads,
     output writes) would otherwise spray 4-byte packets into the same
     queue and delay the first bulk packet by ~2.5us.
  2. ScalarE ACT computes sigmoid(x) per tile — one activation function
     for the bulk pass, so the ACT table stays resident.
  3. VectorE reduces groups of 16 sigmoids with a product (ln prod sigma =
     sum ln sigma; groups of 16 keep the product far above the ~2^-64
     LN-table clamp).
  4. The target entry of each row is fetched by indirect-gather DMA at the
     start; 1/sigmoid(x_t) is appended as one extra product column — its
     ln contributes exactly -ln sigmoid(x_t).  The whole correction chain
     is emitted right after row block 0 so it never lands in the tail.
  5. LN+accumulate per row block over the product columns yields
     sum_c ln sigmoid - ln sigmoid_t; scale by -1/C.  LNs are batched in
     two table visits (rb 0-2 mid-stream, rb 3 in the drain) to halve the
     Sigmoid<->Ln table-swap traffic.  The last column tile of the last
     row block is split so the post-DMA pipeline drain is short.
"""

import sys

sys.path.insert(0, "/opt/trn_rl_repo")

import numpy as np

from concourse import bass, bacc, mybir
import concourse.tile as tile
from concourse.bass_utils import run_bass_kernel_spmd

B, C = 4096, 32000
NCORES = 8
R = B // NCORES  # rows per core
P = 128  # SBUF partitions
NRB = R // P  # row blocks per core

# Tunables (overridable via build_nc kwargs for experiments; the defaults
# are the tuned configuration used for grading).
CT = 2000  # column-tile width
# Product-group size: ln(prod of GRP sigmoids) must stay far above ~2^-64,
# where the ScalarE LN table clamps (HW-measured).  GRP=16 keeps group
# products >= ~1e-12 for randn inputs (>10 sigma of margin); GRP=40 was
# observed to dip below the clamp and corrupt rows.
GRP = 16
PIN_BUFS = 8
PSG_BUFS = 6
TAIL_SPLIT = 240  # width of the final (drain-shortening) tile, multiple of GRP

F32 = mybir.dt.float32
I32 = mybir.dt.int32
SIG = mybir.ActivationFunctionType.Sigmoid
LN = mybir.ActivationFunctionType.Ln


def build_nc(ct=None, grp=None, pin_bufs=None, psg_bufs=None, tail_split=None):
    ct = CT if ct is None else ct
    grp = GRP if grp is None else grp
    pin_bufs = PIN_BUFS if pin_bufs is None else pin_bufs
    psg_bufs = PSG_BUFS if psg_bufs is None else psg_bufs
    tail_split = TAIL_SPLIT if tail_split is None else tail_split

    nct = C // ct  # column tiles per row block
    ng = ct // grp  # product columns per full tile
    ngr = C // grp  # product columns per row block
    assert tail_split % grp == 0 and 0 < tail_split < ct

    nc = bacc.Bacc(None, target_bir_lowering=False)
    pred = nc.declare_dram_parameter("pred", [R, C], F32, isOutput=False)
    gidx = nc.declare_dram_parameter("gidx", [R], I32, isOutput=False)
    out = nc.declare_dram_parameter("out", [R], F32, isOutput=True)

    # Flat [R*C, 1] view of pred for the target-element gather.
    pred_flat = pred[:, :].rearrange("a b -> (a b)")[:, None]

    with tile.TileContext(nc) as tc:
        with (
            tc.tile_pool(name="pin", bufs=pin_bufs) as pin,
            tc.tile_pool(name="psg", bufs=psg_bufs) as psg,
            tc.tile_pool(name="pg", bufs=1) as pg,
            tc.tile_pool(name="pln", bufs=2) as pln,
            tc.tile_pool(name="psm", bufs=2) as psm,
        ):
            # Gather pred[r, target[r]] for all rows: one [P, 1] indirect
            # DMA per row block into a shared [P, NRB] tile.  Index loads
            # ride the gpsimd software queue with the gathers — [P,1]
            # transfers spray 4-byte packets, and a second hardware queue
            # makes the shared DMA engines ring-switch, slowing the bulk
            # stream (HW-measured ~14%).  The memset bounds the damage if
            # a gather ever lands late.
            tv = psm.tile([P, NRB], F32, tag="tv")
            nc.gpsimd.memset(tv[:], 0.0)
            for rb in range(NRB):
                idx_t = psm.tile([P, 1], I32, tag=f"idx{rb}")
                nc.gpsimd.dma_start(
                    out=idx_t[:], in_=gidx[rb * P : (rb + 1) * P, None]
                )
                nc.gpsimd.indirect_dma_start(
                    out=tv[:, rb : rb + 1],
                    out_offset=None,
                    in_=pred_flat,
                    in_offset=bass.IndirectOffsetOnAxis(ap=idx_t[:, :1], axis=0),
                )

            # One product tile per row block: ngr group products plus one
            # correction column holding 1/sigmoid(x_t).
            gt = []
            for rb in range(NRB):
                g_rb = pg.tile([P, ngr + 1], F32, name=f"g{rb}", tag=f"g{rb}")
                gt.append(g_rb)

            # Column-tile widths: full tiles, except the last tile of the
            # last row block is split so the post-DMA drain is short.
            def col_tiles(rb):
                tiles = [(i * ct, ct) for i in range(nct)]
                if rb == NRB - 1:
                    last_off, _ = tiles[-1]
                    tiles[-1] = (last_off, ct - tail_split)
                    tiles.append((last_off + ct - tail_split, tail_split))
                return tiles

            def bulk_tile(rb, c0, w):
                rows = slice(rb * P, (rb + 1) * P)
                t = pin.tile([P, w], F32, name="tin", tag="in")
                nc.sync.dma_start(out=t[:], in_=pred[rows, c0 : c0 + w])
                s = psg.tile([P, w], F32, name="tsig", tag="sig")
                nc.scalar.activation(out=s[:], in_=t[:], func=SIG)
                g0 = c0 // grp
                nc.vector.tensor_reduce(
                    out=gt[rb][:, g0 : g0 + w // grp],
                    in_=s[:].rearrange("p (g k) -> p g k", k=grp),
                    op=mybir.AluOpType.mult,
                    axis=mybir.AxisListType.X,
                )

            def ln_block(rb):
                rows = slice(rb * P, (rb + 1) * P)
                lnout = pln.tile([P, ngr + 1], F32, name="lnout", tag="lnout")
                acc = psm.tile([P, 1], F32, name="acc", tag="acc")
                nc.scalar.activation(
                    out=lnout[:], in_=gt[rb][:], func=LN, accum_out=acc[:]
                )
                o = psm.tile([P, 1], F32, name="o", tag="o")
                nc.scalar.mul(o[:], acc[:], -1.0 / C)
                nc.sync.dma_start(out=out[rows, None], in_=o[:])

            for rb in range(NRB):
                for c0, w in col_tiles(rb):
                    bulk_tile(rb, c0, w)
                if rb == 0:
                    # Correction terms, emitted right after row block 0 so
                    # they are long done before the drain: 1/sigmoid(x_t)
                    # goes into each row block's extra product column (its
                    # ln contributes exactly -ln sigmoid(x_t)).
                    sgt = psm.tile([P, NRB], F32, tag="sgt")
                    nc.scalar.activation(out=sgt[:], in_=tv[:], func=SIG)
                    rec = psm.tile([P, NRB], F32, tag="rec")
                    nc.vector.reciprocal(out=rec[:], in_=sgt[:])
                    with nc.allow_low_precision("correction col; ~1e-7 rel"):
                        for rb2 in range(NRB):
                            nc.vector.tensor_copy(
                                out=gt[rb2][:, ngr : ngr + 1],
                                in_=rec[:, rb2 : rb2 + 1],
                            )
                # Eager LN per row block: the Tile scheduler runs it as
                # soon as gt[rb] is complete; the Sigmoid<->Ln table-swap
                # detour (~4.8us) is absorbed by the 8-deep input pool, so
                # the bulk DMA stream never stalls, and only the last row
                # block's LN lands in the drain.
                ln_block(rb)
    nc.finalize()
    return nc


_NC = None


def _get_nc():
    global _NC
    if _NC is None:
        _NC = build_nc()
    return _NC


def _make_in_maps(pred, target):
    pred = np.ascontiguousarray(np.asarray(pred, dtype=np.float32))
    tgt = np.asarray(target).astype(np.int64)
    in_maps = []
    for c in range(NCORES):
        rs = c * R
        loc_t = tgt[rs : rs + R]
        g = (np.arange(R, dtype=np.int64) * C + loc_t).astype(np.int32)
        in_maps.append({"pred": pred[rs : rs + R], "gidx": g})
    return in_maps


def kernel(pred, target, _trace=False):
    nc = _get_nc()
    in_maps = _make_in_maps(pred, target)
    res = run_bass_kernel_spmd(
        nc, in_maps, core_ids=list(range(NCORES)), trace=_trace
    )
    out = np.concatenate([res.results[i]["out"] for i in range(NCORES)])
    if _trace:
        kernel.last_results = res
    return out.astype(np.float32)


# revision 8
# speedup vs baseline: 1.0654x; 1.0654x over previous
"""Adversarial loss kernel for Trainium2 (8 NeuronCores, data-parallel).

For pred [4096, 32000] f32 and target [4096] int:
    out[b] = -(sum_c log(sigmoid(pred[b,c])) - log(sigmoid(pred[b,target[b]]))) / C

Sharding: pure data parallel over the batch dim — 512 rows per core.

Per-core pipeline (memory-bound problem; ~65.5 MB of pred per core):
  1. DMA [128, CT] tiles of pred into SBUF.
  2. ScalarE ACT computes sigmoid(x) per tile — a single activation
     function for the whole bulk pass, so the ACT table is loaded once.
  3. VectorE reduces groups of 8 sigmoids with a product (ln prod sigma =
     sum ln sigma; groups of 8 keep the product in f32 range).
  4. The target entry of each row is fetched by indirect-gather DMA;
     1/sigmoid(x_t) is appended as one extra product column — its ln
     contributes exactly -ln sigmoid(x_t).
  5. One LN+accumulate activation per row block over the product columns
     yields sum_c ln sigmoid - ln sigmoid_t; scale by -1/C.
"""

import sys

sys.path.insert(0, "/opt/trn_rl_repo")

import numpy as np

from concourse import bass, bacc, mybir
import concourse.tile as tile
from concourse.tile_rust import add_dep_helper
from concourse.bass_utils import run_bass_kernel_spmd

B, C = 4096, 32000
NCORES = 8
R = B // NCORES  # rows per core
P = 128  # SBUF partitions
NRB = R // P  # row blocks per core

# Tunables (overridable via build_nc kwargs for experiments; the defaults
# are the tuned configuration used for grading).
CT = 2000  # column-tile width
# Product-group size: ln(prod of GRP sigmoids) must stay far above ~2^-64,
# where the ScalarE LN table clamps (HW-measured).  GRP=16 keeps group
# products >= ~1e-12 for randn inputs (>10 sigma of margin); GRP=40 was
# observed to dip below the clamp and corrupt rows.
GRP = 16
USE_BF16 = False  # dtype of sigma/product tiles
PIN_BUFS = 8
PSG_BUFS = 6
PIN_LN = False  # force LNs after all sigmoids
DMA_SPLIT = False  # alternate input-DMA issue between sync and scalar HWDGE

F32 = mybir.dt.float32
BF16 = mybir.dt.bfloat16
I32 = mybir.dt.int32
SIG = mybir.ActivationFunctionType.Sigmoid
LN = mybir.ActivationFunctionType.Ln


def build_nc(
    ct=None,
    grp=None,
    use_bf16=None,
    pin_bufs=None,
    psg_bufs=None,
    pin_ln=None,
    dma_split=None,
):
    ct = CT if ct is None else ct
    grp = GRP if grp is None else grp
    use_bf16 = USE_BF16 if use_bf16 is None else use_bf16
    pin_bufs = PIN_BUFS if pin_bufs is None else pin_bufs
    psg_bufs = PSG_BUFS if psg_bufs is None else psg_bufs
    pin_ln = PIN_LN if pin_ln is None else pin_ln
    dma_split = DMA_SPLIT if dma_split is None else dma_split

    nct = C // ct  # column tiles per row block
    ng = ct // grp  # product columns per tile
    ngr = nct * ng  # product columns per row block
    sdt = BF16 if use_bf16 else F32

    nc = bacc.Bacc(None, target_bir_lowering=False)
    pred = nc.declare_dram_parameter("pred", [R, C], F32, isOutput=False)
    gidx = nc.declare_dram_parameter("gidx", [R], I32, isOutput=False)
    out = nc.declare_dram_parameter("out", [R], F32, isOutput=True)

    # Flat [R*C, 1] view of pred for the target-element gather.
    pred_flat = pred[:, :].rearrange("a b -> (a b)")[:, None]

    with tile.TileContext(nc) as tc:
        with (
            tc.tile_pool(name="pin", bufs=pin_bufs) as pin,
            tc.tile_pool(name="psg", bufs=psg_bufs) as psg,
            tc.tile_pool(name="pg", bufs=1) as pg,
            tc.tile_pool(name="pln", bufs=2) as pln,
            tc.tile_pool(name="psm", bufs=2) as psm,
        ):
            # Gather pred[r, target[r]] for all rows: one [P, 1] indirect
            # DMA per row block into a shared [P, NRB] tile.  The memset
            # bounds the damage if a gather ever lands late.
            tv = psm.tile([P, NRB], F32, tag="tv")
            nc.gpsimd.memset(tv[:], 0.0)
            for rb in range(NRB):
                idx_t = psm.tile([P, 1], I32, tag=f"idx{rb}")
                nc.sync.dma_start(
                    out=idx_t[:], in_=gidx[rb * P : (rb + 1) * P, None]
                )
                nc.gpsimd.indirect_dma_start(
                    out=tv[:, rb : rb + 1],
                    out_offset=None,
                    in_=pred_flat,
                    in_offset=bass.IndirectOffsetOnAxis(ap=idx_t[:, :1], axis=0),
                )

            # One product tile per row block: ngr group products plus one
            # correction column holding 1/sigmoid(x_t).
            gt = []
            for rb in range(NRB):
                g_rb = pg.tile([P, ngr + 1], sdt, tag=f"g{rb}")
                gt.append(g_rb)

            last_sig = None
            for rb in range(NRB):
                rows = slice(rb * P, (rb + 1) * P)
                for cti in range(nct):
                    t = pin.tile([P, ct], F32, tag="in")
                    dma_eng = (
                        nc.scalar if (dma_split and cti % 2) else nc.sync
                    )
                    dma_eng.dma_start(
                        out=t[:], in_=pred[rows, cti * ct : (cti + 1) * ct]
                    )
                    s = psg.tile([P, ct], sdt, tag="sig")
                    last_sig = nc.scalar.activation(out=s[:], in_=t[:], func=SIG)
                    with nc.allow_low_precision(
                        "sigmoid-product groups; ln(prod) error averages "
                        "out over 32000 summed terms (~1e-5 rel on the loss)"
                    ):
                        nc.vector.tensor_reduce(
                            out=gt[rb][:, cti * ng : (cti + 1) * ng],
                            in_=s[:].rearrange("p (g k) -> p g k", k=grp),
                            op=mybir.AluOpType.mult,
                            axis=mybir.AxisListType.X,
                        )

            # Correction terms, emitted after the bulk loop so the gathers
            # above have the whole bulk pass of slack before sigma(x_t) is
            # consumed: 1/sigmoid(x_t) goes into each row block's extra
            # product column (its ln contributes exactly -ln sigmoid(x_t)).
            sgt = psm.tile([P, NRB], F32, tag="sgt")
            nc.scalar.activation(out=sgt[:], in_=tv[:], func=SIG)
            rec = psm.tile([P, NRB], F32, tag="rec")
            nc.vector.reciprocal(out=rec[:], in_=sgt[:])
            with nc.allow_low_precision("correction column cast; ~1e-7 rel"):
                for rb in range(NRB):
                    nc.vector.tensor_copy(
                        out=gt[rb][:, ngr : ngr + 1], in_=rec[:, rb : rb + 1]
                    )

            # ln of all product columns, accumulated per row -> the loss.
            for rb in range(NRB):
                rows = slice(rb * P, (rb + 1) * P)
                lnout = pln.tile([P, ngr + 1], sdt, tag="lnout")
                acc = psm.tile([P, 1], F32, tag="acc")
                ln_inst = nc.scalar.activation(
                    out=lnout[:], in_=gt[rb][:], func=LN, accum_out=acc[:]
                )
                if pin_ln:
                    add_dep_helper(
                        ln_inst.ins,
                        last_sig.ins,
                        reason="batch LNs after sigmoids",
                    )
                o = psm.tile([P, 1], F32, tag="o")
                nc.scalar.mul(o[:], acc[:], -1.0 / C)
                nc.sync.dma_start(out=out[rows, None], in_=o[:])
    nc.finalize()
    return nc


_NC = None


def _get_nc():
    global _NC
    if _NC is None:
        _NC = build_nc()
    return _NC


def _make_in_maps(pred, target):
    pred = np.ascontiguousarray(np.asarray(pred, dtype=np.float32))
    tgt = np.asarray(target).astype(np.int64)
    in_maps = []
    for c in range(NCORES):
        rs = c * R
        loc_t = tgt[rs : rs + R]
        g = (np.arange(R, dtype=np.int64) * C + loc_t).astype(np.int32)
        in_maps.append({"pred": pred[rs : rs + R], "gidx": g})
    return in_maps


def kernel(pred, target, _trace=False):
    nc = _get_nc()
    in_maps = _make_in_maps(pred, target)
    res = run_bass_kernel_spmd(
        nc, in_maps, core_ids=list(range(NCORES)), trace=_trace
    )
    out = np.concatenate([res.results[i]["out"] for i in range(NCORES)])
    if _trace:
        kernel.last_results = res
    return out.astype(np.float32)



# revision 10
# speedup vs baseline: 1.1708x; 1.0989x over previous
"""Adversarial loss kernel for Trainium2 (8 NeuronCores, data-parallel).

For pred [4096, 32000] f32 and target [4096] int:
    out[b] = -(sum_c log(sigmoid(pred[b,c])) - log(sigmoid(pred[b,target[b]]))) / C

Sharding: pure data parallel over the batch dim — 512 rows per core.

Per-core pipeline (memory-bound problem; ~65.5 MB of pred per core):
  1. DMA [128, 8000] tiles of pred into SBUF via the sync HWDGE queue —
     and ONLY bulk tiles on that queue: [P,1]-shaped transfers spray
     4-byte packets, and on a second hardware queue they make the shared
     DMA engines ring-switch, slowing the bulk stream (HW-measured ~14%).
     8000-wide tiles produce 32KB packets, which measured ~7% higher DMA
     busy-rate than the 8KB packets of 2000-wide tiles, and amortize the
     per-instruction bubbles of ScalarE (224 cyc) and VectorE (58 cyc).
     The first two bulk tiles are emitted before the index loads so the
     queue starts streaming immediately.
  2. ScalarE ACT computes sigmoid(x) per tile — one table for the bulk.
  3. VectorE reduces groups of 16 sigmoids with a product (ln prod sigma =
     sum ln sigma; groups of 16 stay far above the ~2^-64 LN-table clamp).
     tensor_reduce has only a 1x uop (~1 elem/cycle regardless of dtype),
     so VectorE runs ~146us — under DMA's ~157-165us but in-order, which
     is why the tail tiling below matters.
  4. The target entry of each row comes by indirect-gather DMA (indices
     via sync so the gather's ucode is ordered behind a HW semaphore;
     a software-DGE index load was observed to race).  The gathered
     values are consumed only after row block 2 — the gather completion
     has no reliable sync, so it gets a ~100us slack window; the memset
     bounds the damage if one ever lands late.  1/sigmoid(x_t) goes into
     one extra product column — its ln contributes exactly
     -ln sigmoid(x_t).
  5. LN+accumulate per row block over the product columns, scaled by
     -1/C.  The Sigmoid<->Ln table-swap detours are absorbed by the input
     pool.  The last row block uses progressively finer tiles and a split
     LN (part A emitted after the last sigmoid, overlapping the final
     reduces) so the post-DMA drain is ~8us instead of ~24.
"""

import sys

sys.path.insert(0, "/opt/trn_rl_repo")

import numpy as np

from concourse import bass, bacc, mybir
import concourse.tile as tile
from concourse.bass_utils import run_bass_kernel_spmd

B, C = 4096, 32000
NCORES = 8
R = B // NCORES  # rows per core
P = 128  # SBUF partitions
NRB = R // P  # row blocks per core

CT = 8000  # column-tile width
GRP = 16  # product-group size (see step 3 above)
PIN_BUFS = 3
PSG_BUFS = 3
# Last row block uses progressively finer tiles: VectorE's reduce is
# in-order and ~1 elem/cycle, so a wide reduce in flight at last-byte
# serializes the drain behind it.
TAIL_TILES = (4000, 4000, 4000, 4000, 4000, 4000, 4000, 2400, 800, 480, 320)
LN3_SPLIT = 1900  # group boundary: LN[3] part A covers [0:1900), emitted
# after the last sigmoid so it overlaps the final small reduces; only a
# 101-column part B serializes behind the last reduce.
PRE_TILES = 2  # bulk tiles emitted before the index loads
CORR_AFTER_RB = 2  # emit the correction chain after this row block

F32 = mybir.dt.float32
BF16 = mybir.dt.bfloat16
I32 = mybir.dt.int32
SIG = mybir.ActivationFunctionType.Sigmoid
LN = mybir.ActivationFunctionType.Ln


def build_nc(
    ct=None,
    grp=None,
    pin_bufs=None,
    psg_bufs=None,
    tail_tiles=None,
    pre_tiles=None,
    corr_after_rb=None,
    alt_reduce=False,
):
    ct = CT if ct is None else ct
    grp = GRP if grp is None else grp
    pin_bufs = PIN_BUFS if pin_bufs is None else pin_bufs
    psg_bufs = PSG_BUFS if psg_bufs is None else psg_bufs
    tail_tiles = TAIL_TILES if tail_tiles is None else tail_tiles
    pre_tiles = PRE_TILES if pre_tiles is None else pre_tiles
    corr_after_rb = CORR_AFTER_RB if corr_after_rb is None else corr_after_rb

    nct = C // ct  # column tiles per row block
    ngr = C // grp  # product columns per row block
    assert sum(tail_tiles) == C and all(w % grp == 0 for w in tail_tiles)

    nc = bacc.Bacc(None, target_bir_lowering=False)
    pred = nc.declare_dram_parameter("pred", [R, C], F32, isOutput=False)
    gidx = nc.declare_dram_parameter("gidx", [R], I32, isOutput=False)
    out = nc.declare_dram_parameter("out", [R], F32, isOutput=True)

    # Flat [R*C, 1] view of pred for the target-element gather.
    pred_flat = pred[:, :].rearrange("a b -> (a b)")[:, None]

    with tile.TileContext(nc) as tc:
        with (
            tc.tile_pool(name="pin", bufs=pin_bufs) as pin,
            tc.tile_pool(name="psg", bufs=psg_bufs) as psg,
            tc.tile_pool(name="pg", bufs=1) as pg,
            tc.tile_pool(name="pln", bufs=2) as pln,
            tc.tile_pool(name="psm", bufs=2) as psm,
        ):
            # One product tile per row block: ngr group products plus one
            # correction column holding 1/sigmoid(x_t).
            gt = []
            for rb in range(NRB):
                g_rb = pg.tile([P, ngr + 1], F32, name=f"g{rb}", tag=f"g{rb}")
                gt.append(g_rb)
            tv = psm.tile([P, NRB], F32, tag="tv")

            def col_tiles(rb):
                if rb == NRB - 1:
                    tiles, off = [], 0
                    for w in tail_tiles:
                        tiles.append((off, w))
                        off += w
                    return tiles
                return [(i * ct, ct) for i in range(nct)]

            tile_no = [0]

            def bulk_tile(rb, c0, w):
                rows = slice(rb * P, (rb + 1) * P)
                t = pin.tile([P, w], F32, name="tin", tag="in")
                nc.sync.dma_start(out=t[:], in_=pred[rows, c0 : c0 + w])
                s = psg.tile([P, w], BF16, name="tsig", tag="sig")
                nc.scalar.activation(out=s[:], in_=t[:], func=SIG)
                g0 = c0 // grp
                # Alternate product reduces between VectorE and the Pool
                # engine — each alone is nearly co-critical with DMA.
                red_eng = (
                    nc.gpsimd if (alt_reduce and tile_no[0] % 2) else nc.vector
                )
                tile_no[0] += 1
                with nc.allow_low_precision(
                    "bf16 sigmoid-product groups; ln(prod) error averages "
                    "out over 32000 summed terms (~1e-5 rel on the loss)"
                ):
                    red_eng.tensor_reduce(
                        out=gt[rb][:, g0 : g0 + w // grp],
                        in_=s[:].rearrange("p (g k) -> p g k", k=grp),
                        op=mybir.AluOpType.mult,
                        axis=mybir.AxisListType.X,
                    )

            def gather_block():
                # Index loads on sync (hardware queue — its completion
                # semaphore orders the gather's ucode correctly; a
                # software-DGE index load was observed to race).  Gathers
                # on the gpsimd software queue.
                nc.gpsimd.memset(tv[:], 0.0)
                for rb in range(NRB):
                    idx_t = psm.tile([P, 1], I32, tag=f"idx{rb}")
                    nc.sync.dma_start(
                        out=idx_t[:], in_=gidx[rb * P : (rb + 1) * P, None]
                    )
                    nc.gpsimd.indirect_dma_start(
                        out=tv[:, rb : rb + 1],
                        out_offset=None,
                        in_=pred_flat,
                        in_offset=bass.IndirectOffsetOnAxis(
                            ap=idx_t[:, :1], axis=0
                        ),
                    )

            def correction_block():
                # 1/sigmoid(x_t) into each row block's extra product
                # column; its ln contributes exactly -ln sigmoid(x_t).
                sgt = psm.tile([P, NRB], F32, tag="sgt")
                nc.scalar.activation(out=sgt[:], in_=tv[:], func=SIG)
                rec = psm.tile([P, NRB], F32, tag="rec")
                nc.vector.reciprocal(out=rec[:], in_=sgt[:])
                with nc.allow_low_precision("correction col; ~1e-7 rel"):
                    for rb2 in range(NRB):
                        nc.vector.tensor_copy(
                            out=gt[rb2][:, ngr : ngr + 1],
                            in_=rec[:, rb2 : rb2 + 1],
                        )

            def ln_block(rb, split=None):
                rows = slice(rb * P, (rb + 1) * P)
                lnout = pln.tile([P, ngr + 1], F32, name="lnout", tag="lnout")
                if split is None:
                    acc = psm.tile([P, 1], F32, name="acc", tag="acc")
                    nc.scalar.activation(
                        out=lnout[:], in_=gt[rb][:], func=LN, accum_out=acc[:]
                    )
                else:
                    acc_a = psm.tile([P, 1], F32, name="acc_a", tag="acc_a")
                    nc.scalar.activation(
                        out=lnout[:, :split],
                        in_=gt[rb][:, :split],
                        func=LN,
                        accum_out=acc_a[:],
                    )
                    acc_b = psm.tile([P, 1], F32, name="acc_b", tag="acc_b")
                    nc.scalar.activation(
                        out=lnout[:, split:],
                        in_=gt[rb][:, split:],
                        func=LN,
                        accum_out=acc_b[:],
                    )
                    acc = psm.tile([P, 1], F32, name="acc", tag="acc")
                    nc.scalar.add(acc[:], acc_b[:], acc_a[:, :1])
                o = psm.tile([P, 1], F32, name="o", tag="o")
                nc.scalar.mul(o[:], acc[:], -1.0 / C)
                nc.sync.dma_start(out=out[rows, None], in_=o[:])

            ln_done = [False] * NRB
            for rb in range(NRB):
                for i, (c0, w) in enumerate(col_tiles(rb)):
                    bulk_tile(rb, c0, w)
                    if rb == 0 and i == pre_tiles - 1:
                        gather_block()
                if rb == corr_after_rb:
                    correction_block()
                if rb >= corr_after_rb:
                    for rb2 in range(rb + 1):
                        if not ln_done[rb2]:
                            ln_block(
                                rb2,
                                split=LN3_SPLIT if rb2 == NRB - 1 else None,
                            )
                            ln_done[rb2] = True
    nc.finalize()
    return nc


_NC = None


def _get_nc():
    global _NC
    if _NC is None:
        _NC = build_nc()
    return _NC


def _make_in_maps(pred, target):
    pred = np.ascontiguousarray(np.asarray(pred, dtype=np.float32))
    tgt = np.asarray(target).astype(np.int64)
    in_maps = []
    for c in range(NCORES):
        rs = c * R
        loc_t = tgt[rs : rs + R]
        g = (np.arange(R, dtype=np.int64) * C + loc_t).astype(np.int32)
        in_maps.append({"pred": pred[rs : rs + R], "gidx": g})
    return in_maps


def kernel(pred, target, _trace=False):
    nc = _get_nc()
    in_maps = _make_in_maps(pred, target)
    res = run_bass_kernel_spmd(
        nc, in_maps, core_ids=list(range(NCORES)), trace=_trace
    )
    out = np.concatenate([res.results[i]["out"] for i in range(NCORES)])
    if _trace:
        kernel.last_results = res
    return out.astype(np.float32)
